# revision 5
# baseline (speedup 1.0000x reference)
"""Trainium2 Bass kernel for nn_BD dense MLP (block-diagonal hidden layers).

Network: x[B,64] -> relu(x@W_in)[B,32] -> 4x relu(h@(mask*W_h))[B,32]
         -> h@(mask*W_out)[B,24]

Strategy (pure data parallel over 8 cores, B=1048576, R=131072 rows/core):
 - x loaded batch-major contiguously; DVE 32x32 block-transpose flips each
   [32 batch x 32 feat] block to feature-major. The resulting batch
   permutation is undone by the output DMA access pattern.
 - All matmuls run feature-major: weights stationary (lhsT), activations
   moving (rhs, N=512). 4 batch chunks sit at partition groups 0..3 and use
   diagonal PE tile positions (rows 32c, cols 32c) so 4 matmuls stream
   concurrently in different 32x32 sub-arrays.
 - ReLU fused into the PSUM->SBUF move on ScalarE/VectorE at full 128
   partitions.
 - Output written padded [R,32]; host strips to 24 cols.
"""

import sys

import numpy as np

if "/opt/trn_rl_repo" not in sys.path:
    sys.path.insert(0, "/opt/trn_rl_repo")

N_CORES = 8
B_FULL = 1048576
R = B_FULL // N_CORES  # rows per core
SLAB = 4096  # rows per pipeline iteration
F32 = None  # set after import


def build_nc(rows=R, act_split=(True, True, True, True, False)):
    """Build the single-core SPMD Bass graph.

    act_split[l]: True -> relu on ScalarE, False -> relu on VectorE.
    """
    import concourse.bass as bass  # noqa: F401
    import concourse.mybir as mybir
    from concourse import bacc, tile

    f32 = mybir.dt.float32
    nc = bacc.Bacc(None)

    x_ext = nc.declare_dram_parameter("x", [rows, 64], f32, isOutput=False)
    w1_ext = nc.declare_dram_parameter("w1", [128, 64], f32, isOutput=False)
    wh_ext = nc.declare_dram_parameter("wh", [128, 128], f32, isOutput=False)
    wo_ext = nc.declare_dram_parameter("wo", [128, 32], f32, isOutput=False)
    out_ext = nc.declare_dram_parameter("out", [rows, 32], f32, isOutput=True)

    n_slabs = rows // SLAB
    # x row r = s*4096 + p*32 + n  (p = SBUF partition, n = 0..31)
    x_r = x_ext.rearrange("(s p n) f -> s p (n f)", p=128, n=32)
    # out row r = s*4096 + pg*1024 + b*32 + n ; partition = 32*pg + b
    o_r = out_ext.rearrange("(s pg b n) c -> s (pg b) (n c)", pg=4, b=32, n=32)

    Relu = mybir.ActivationFunctionType.Relu

    with tile.TileContext(nc) as tc:
        with (
            tc.tile_pool(name="const", bufs=1) as cpool,
            tc.tile_pool(name="xin", bufs=3) as xpool,
            tc.tile_pool(name="xt", bufs=2) as xtpool,
            tc.tile_pool(name="h", bufs=3) as hpool,
            tc.tile_pool(name="hps", bufs=3, space="PSUM") as hpsp,
            tc.tile_pool(name="ops", bufs=1, space="PSUM") as opsp,
            tc.tile_pool(name="ob", bufs=2) as opool,
            tc.tile_pool(name="ot", bufs=2) as otpool,
        ):
            w1 = cpool.tile([128, 64], f32, tag="w1")
            nc.sync.dma_start(w1[:, :], w1_ext[:, :])
            wh = cpool.tile([128, 128], f32, tag="wh")
            nc.sync.dma_start(wh[:, :], wh_ext[:, :])
            wo = cpool.tile([128, 32], f32, tag="wo")
            nc.sync.dma_start(wo[:, :], wo_ext[:, :])

            for s in range(n_slabs):
                x_sb = xpool.tile([128, 2048], f32, tag="x")
                nc.sync.dma_start(x_sb[:, :], x_r[s])

                xt = xtpool.tile([128, 2048], f32, tag="xt")
                nc.vector.transpose(xt[:, :], x_sb[:, :])
                # xt[32*pg + f_, 64*n + 32*fb + b] = x[row(pg,b,n), 32*fb + f_]
                xt_v = xt[:, :].rearrange("p (n fb b) -> p n fb b", fb=2, b=32)

                # Layer 1: 64 -> 32, two accumulated K=32 pieces
                ps = hpsp.tile([128, 1024], f32, tag="hps")
                for pg in range(4):
                    sl = slice(32 * pg, 32 * pg + 32)
                    for hh in range(2):
                        for fb in range(2):
                            nc.tensor.matmul(
                                ps[sl, 512 * hh : 512 * hh + 512],
                                lhsT=w1[sl, 32 * fb : 32 * fb + 32],
                                rhs=xt_v[sl, 16 * hh : 16 * hh + 16, fb, :],
                                start=(fb == 0),
                                stop=(fb == 1),
                                tile_position=(32 * pg, 32 * pg),
                            )
                hprev = hpool.tile([128, 1024], f32, tag="h")
                if act_split[0]:
                    nc.scalar.activation(hprev[:, :], ps[:, :], Relu)
                else:
                    nc.vector.tensor_scalar_max(hprev[:, :], ps[:, :], 0.0)

                # Layers 2..5: block-diag 32 -> 32
                for l in range(4):
                    ps = hpsp.tile([128, 1024], f32, tag="hps")
                    for pg in range(4):
                        sl = slice(32 * pg, 32 * pg + 32)
                        for hh in range(2):
                            nc.tensor.matmul(
                                ps[sl, 512 * hh : 512 * hh + 512],
                                lhsT=wh[sl, 32 * l : 32 * l + 32],
                                rhs=hprev[sl, 512 * hh : 512 * hh + 512],
                                start=True,
                                stop=True,
                                tile_position=(32 * pg, 32 * pg),
                            )
                    hnew = hpool.tile([128, 1024], f32, tag="h")
                    if act_split[l + 1]:
                        nc.scalar.activation(hnew[:, :], ps[:, :], Relu)
                    else:
                        nc.vector.tensor_scalar_max(hnew[:, :], ps[:, :], 0.0)
                    hprev = hnew

                # Layer 6: block-diag 32 -> 24 (padded to 32 with zero cols)
                ops_t = opsp.tile([128, 1024], f32, tag="ops")
                for pg in range(4):
                    sl = slice(32 * pg, 32 * pg + 32)
                    for hh in range(2):
                        nc.tensor.matmul(
                            ops_t[sl, 512 * hh : 512 * hh + 512],
                            lhsT=wo[sl, :],
                            rhs=hprev[sl, 512 * hh : 512 * hh + 512],
                            start=True,
                            stop=True,
                            tile_position=(32 * pg, 32 * pg),
                        )
                o_sb = opool.tile([128, 1024], f32, tag="ob")
                nc.scalar.copy(o_sb[:, :], ops_t[:, :])
                ot = otpool.tile([128, 1024], f32, tag="ot")
                nc.vector.transpose(ot[:, :], o_sb[:, :])
                nc.sync.dma_start(o_r[s], ot[:, :])
    nc.compile()
    return nc


def prep_weights(input_weight, hidden_weights, output_weights):
    hid_filter = np.kron(np.eye(4, dtype=np.float32), np.ones((8, 8), np.float32))
    out_filter = np.kron(np.eye(8, dtype=np.float32), np.ones((4, 3), np.float32))
    whm = hid_filter[None] * np.asarray(hidden_weights, np.float32)  # [4,32,32]
    wom = out_filter * np.asarray(output_weights, np.float32)  # [32,24]
    w_in = np.asarray(input_weight, np.float32)  # [64,32]

    w1 = np.zeros((128, 64), np.float32)
    wh = np.zeros((128, 128), np.float32)
    wo = np.zeros((128, 32), np.float32)
    for pg in range(4):
        for fb in range(2):
            w1[32 * pg : 32 * pg + 32, 32 * fb : 32 * fb + 32] = w_in[
                32 * fb : 32 * fb + 32, :
            ]
        for l in range(4):
            wh[32 * pg : 32 * pg + 32, 32 * l : 32 * l + 32] = whm[l]
        wo[32 * pg : 32 * pg + 32, :24] = wom
    return w1, wh, wo


def kernel(x, input_weight, hidden_weights, output_weights):
    from concourse.bass_utils import run_bass_kernel_spmd

    x = np.ascontiguousarray(np.asarray(x, np.float32))
    w1, wh, wo = prep_weights(input_weight, hidden_weights, output_weights)

    nc = build_nc(R)
    shards = x.reshape(N_CORES, R, 64)
    in_maps = [
        {"x": shards[i], "w1": w1, "wh": wh, "wo": wo} for i in range(N_CORES)
    ]
    res = run_bass_kernel_spmd(nc, in_maps, core_ids=list(range(N_CORES)))
    outs = [np.asarray(res.results[i]["out"])[:, :24] for i in range(N_CORES)]
    return np.concatenate(outs, axis=0)


# revision 10
# speedup vs baseline: 1.8596x; 1.8596x over previous
"""Trainium2 Bass kernel for nn_BD dense MLP (block-diagonal hidden layers).

Network: x[B,64] -> relu(x@W_in)[B,32] -> 4x relu(h@(mask*W_h))[B,32]
         -> h@(mask*W_out)[B,24]

Strategy (pure data parallel over 8 cores, B=1048576, R=131072 rows/core):
 - x loaded batch-major contiguously; DVE 32x32 block-transpose flips each
   [32 batch x 32 feat] block to feature-major. The resulting batch
   permutation is undone by the output DMA access pattern.
 - All matmuls run feature-major: weights stationary (lhsT), activations
   moving (rhs, N=512). 4 batch chunks sit at partition groups 0..3 and use
   diagonal PE tile positions (rows 32c, cols 32c) so 4 matmuls stream
   concurrently in different 32x32 sub-arrays.
 - ReLU fused into the PSUM->SBUF move on ScalarE/VectorE at full 128
   partitions.
 - Output written padded [R,32]; host strips to 24 cols.
"""

import sys

import numpy as np

if "/opt/trn_rl_repo" not in sys.path:
    sys.path.insert(0, "/opt/trn_rl_repo")

N_CORES = 8
B_FULL = 1048576
R = B_FULL // N_CORES  # rows per core
SLAB = 4096  # rows per pipeline iteration
F32 = None  # set after import


def build_nc(rows=R, act_split=(True, True, True, True, False)):
    """Build the single-core SPMD Bass graph.

    act_split[l]: True -> relu on ScalarE, False -> relu on VectorE.
    """
    import concourse.bass as bass  # noqa: F401
    import concourse.mybir as mybir
    from concourse import bacc, tile

    f32 = mybir.dt.float32
    bf16 = mybir.dt.bfloat16
    nc = bacc.Bacc(None)

    x_ext = nc.declare_dram_parameter("x", [rows, 64], f32, isOutput=False)
    w1_ext = nc.declare_dram_parameter("w1", [128, 64], f32, isOutput=False)
    wh_ext = nc.declare_dram_parameter("wh", [128, 128], f32, isOutput=False)
    wo_ext = nc.declare_dram_parameter("wo", [128, 32], f32, isOutput=False)
    out_ext = nc.declare_dram_parameter("out", [rows, 32], f32, isOutput=True)

    n_slabs = rows // SLAB
    # x row r = s*4096 + p*32 + n  (p = SBUF partition, n = 0..31)
    x_r = x_ext.rearrange("(s p n) f -> s p (n f)", p=128, n=32)
    # out row r = s*4096 + pg*1024 + b*32 + n ; partition = 32*pg + b
    o_r = out_ext.rearrange("(s pg b n) c -> s (pg b) (n c)", pg=4, b=32, n=32)

    Relu = mybir.ActivationFunctionType.Relu

    with tile.TileContext(nc) as tc:
        with (
            tc.tile_pool(name="const", bufs=1) as cpool,
            tc.tile_pool(name="xin", bufs=3) as xpool,
            tc.tile_pool(name="xt", bufs=2) as xtpool,
            tc.tile_pool(name="h", bufs=3) as hpool,
            tc.tile_pool(name="hps", bufs=3, space="PSUM") as hpsp,
            tc.tile_pool(name="ops", bufs=1, space="PSUM") as opsp,
            tc.tile_pool(name="ob", bufs=2) as opool,
            tc.tile_pool(name="ot", bufs=2) as otpool,
        ):
            # weights cast f32 -> bf16 during SWDGE DMA
            w1 = cpool.tile([128, 64], bf16, tag="w1")
            nc.gpsimd.dma_start(w1[:, :], w1_ext[:, :])
            wh = cpool.tile([128, 128], bf16, tag="wh")
            nc.gpsimd.dma_start(wh[:, :], wh_ext[:, :])
            wo = cpool.tile([128, 32], bf16, tag="wo")
            nc.gpsimd.dma_start(wo[:, :], wo_ext[:, :])

            for s in range(n_slabs):
                x_sb = xpool.tile([128, 2048], bf16, tag="x")
                nc.gpsimd.dma_start(x_sb[:, :], x_r[s])

                xt = xtpool.tile([128, 2048], bf16, tag="xt")
                nc.vector.transpose(xt[:, :], x_sb[:, :])
                # xt[32*pg + f_, 64*n + 32*fb + b] = x[row(pg,b,n), 32*fb + f_]
                xt_v = xt[:, :].rearrange("p (n fb b) -> p n fb b", fb=2, b=32)

                # Layer 1: 64 -> 32, two accumulated K=32 pieces
                ps = hpsp.tile([128, 1024], f32, tag="hps")
                for pg in range(4):
                    sl = slice(32 * pg, 32 * pg + 32)
                    for hh in range(2):
                        for fb in range(2):
                            nc.tensor.matmul(
                                ps[sl, 512 * hh : 512 * hh + 512],
                                lhsT=w1[sl, 32 * fb : 32 * fb + 32],
                                rhs=xt_v[sl, 16 * hh : 16 * hh + 16, fb, :],
                                start=(fb == 0),
                                stop=(fb == 1),
                                tile_position=(32 * pg, 32 * pg),
                            )
                hprev = hpool.tile([128, 1024], bf16, tag="h")
                if act_split[0]:
                    nc.scalar.activation(hprev[:, :], ps[:, :], Relu)
                else:
                    nc.vector.tensor_scalar_max(hprev[:, :], ps[:, :], 0.0)

                # Layers 2..5: block-diag 32 -> 32
                for l in range(4):
                    ps = hpsp.tile([128, 1024], f32, tag="hps")
                    for pg in range(4):
                        sl = slice(32 * pg, 32 * pg + 32)
                        for hh in range(2):
                            nc.tensor.matmul(
                                ps[sl, 512 * hh : 512 * hh + 512],
                                lhsT=wh[sl, 32 * l : 32 * l + 32],
                                rhs=hprev[sl, 512 * hh : 512 * hh + 512],
                                start=True,
                                stop=True,
                                tile_position=(32 * pg, 32 * pg),
                            )
                    hnew = hpool.tile([128, 1024], bf16, tag="h")
                    if act_split[l + 1]:
                        nc.scalar.activation(hnew[:, :], ps[:, :], Relu)
                    else:
                        nc.vector.tensor_scalar_max(hnew[:, :], ps[:, :], 0.0)
                    hprev = hnew

                # Layer 6: block-diag 32 -> 24 (padded to 32 with zero cols)
                ops_t = opsp.tile([128, 1024], f32, tag="ops")
                for pg in range(4):
                    sl = slice(32 * pg, 32 * pg + 32)
                    for hh in range(2):
                        nc.tensor.matmul(
                            ops_t[sl, 512 * hh : 512 * hh + 512],
                            lhsT=wo[sl, :],
                            rhs=hprev[sl, 512 * hh : 512 * hh + 512],
                            start=True,
                            stop=True,
                            tile_position=(32 * pg, 32 * pg),
                        )
                o_sb = opool.tile([128, 1024], bf16, tag="ob")
                nc.scalar.copy(o_sb[:, :], ops_t[:, :])
                ot = otpool.tile([128, 1024], bf16, tag="ot")
                nc.vector.transpose(ot[:, :], o_sb[:, :])
                nc.gpsimd.dma_start(o_r[s], ot[:, :])
    nc.compile()
    return nc


def prep_weights(input_weight, hidden_weights, output_weights):
    hid_filter = np.kron(np.eye(4, dtype=np.float32), np.ones((8, 8), np.float32))
    out_filter = np.kron(np.eye(8, dtype=np.float32), np.ones((4, 3), np.float32))
    whm = hid_filter[None] * np.asarray(hidden_weights, np.float32)  # [4,32,32]
    wom = out_filter * np.asarray(output_weights, np.float32)  # [32,24]
    w_in = np.asarray(input_weight, np.float32)  # [64,32]

    w1 = np.zeros((128, 64), np.float32)
    wh = np.zeros((128, 128), np.float32)
    wo = np.zeros((128, 32), np.float32)
    for pg in range(4):
        for fb in range(2):
            w1[32 * pg : 32 * pg + 32, 32 * fb : 32 * fb + 32] = w_in[
                32 * fb : 32 * fb + 32, :
            ]
        for l in range(4):
            wh[32 * pg : 32 * pg + 32, 32 * l : 32 * l + 32] = whm[l]
        wo[32 * pg : 32 * pg + 32, :24] = wom
    return w1, wh, wo


def kernel(x, input_weight, hidden_weights, output_weights):
    from concourse.bass_utils import run_bass_kernel_spmd

    x = np.ascontiguousarray(np.asarray(x, np.float32))
    w1, wh, wo = prep_weights(input_weight, hidden_weights, output_weights)

    nc = build_nc(R)
    shards = x.reshape(N_CORES, R, 64)
    in_maps = [
        {"x": shards[i], "w1": w1, "wh": wh, "wo": wo} for i in range(N_CORES)
    ]
    res = run_bass_kernel_spmd(nc, in_maps, core_ids=list(range(N_CORES)))
    outs = [np.asarray(res.results[i]["out"])[:, :24] for i in range(N_CORES)]
    return np.concatenate(outs, axis=0)


# revision 17
# speedup vs baseline: 2.4996x; 1.3441x over previous
"""Trainium2 Bass kernel for nn_BD dense MLP (block-diagonal hidden layers).

Network: x[B,64] -> relu(x@W_in)[B,32] -> 4x relu(h@(mask*W_h))[B,32]
         -> h@(mask*W_out)[B,24]

Strategy (pure data parallel over 8 cores, B=1048576, R=131072 rows/core):
 - x loaded batch-major contiguously; DVE 32x32 block-transpose flips each
   [32 batch x 32 feat] block to feature-major. The resulting batch
   permutation is undone by the output DMA access pattern.
 - All matmuls run feature-major: weights stationary (lhsT), activations
   moving (rhs, N=512). 4 batch chunks sit at partition groups 0..3 and use
   diagonal PE tile positions (rows 32c, cols 32c) so 4 matmuls stream
   concurrently in different 32x32 sub-arrays.
 - ReLU fused into the PSUM->SBUF move on ScalarE/VectorE at full 128
   partitions.
 - Output written padded [R,32]; host strips to 24 cols.
"""

import sys

import numpy as np

if "/opt/trn_rl_repo" not in sys.path:
    sys.path.insert(0, "/opt/trn_rl_repo")

N_CORES = 8
B_FULL = 1048576
R = B_FULL // N_CORES  # rows per core
SLAB = 4096  # rows per pipeline iteration
F32 = None  # set after import


def build_nc(rows=R, act_split=(True, True, True, True, False)):
    """Build the single-core SPMD Bass graph.

    act_split[l]: True -> relu on ScalarE, False -> relu on VectorE.
    """
    import concourse.bass as bass  # noqa: F401
    import concourse.mybir as mybir
    from concourse import bacc, tile

    f32 = mybir.dt.float32
    bf16 = mybir.dt.bfloat16
    nc = bacc.Bacc(None)

    x_ext = nc.declare_dram_parameter("x", [rows, 64], f32, isOutput=False)
    # 7 block-diagonal 128x128 stationaries: L1 fb0, L1 fb1, L2..L5, L6
    wbd_ext = nc.declare_dram_parameter("wbd", [128, 896], f32, isOutput=False)
    out_ext = nc.declare_dram_parameter("out", [rows, 32], f32, isOutput=True)

    n_slabs = rows // SLAB
    # x row r = s*4096 + p*32 + n  (p = SBUF partition, n = 0..31)
    x_r = x_ext.rearrange("(s p n) f -> s p (n f)", p=128, n=32)
    # out row r = s*4096 + pg*1024 + b*32 + n ; partition = 32*pg + b
    o_r = out_ext.rearrange("(s pg b n) c -> s (pg b) (n c)", pg=4, b=32, n=32)

    Relu = mybir.ActivationFunctionType.Relu

    with tile.TileContext(nc) as tc:
        with (
            tc.tile_pool(name="const", bufs=1) as cpool,
            tc.tile_pool(name="xin", bufs=3) as xpool,
            tc.tile_pool(name="xt", bufs=2) as xtpool,
            tc.tile_pool(name="h", bufs=3) as hpool,
            tc.tile_pool(name="hps", bufs=3, space="PSUM") as hpsp,
            tc.tile_pool(name="ops", bufs=1, space="PSUM") as opsp,
            tc.tile_pool(name="ob", bufs=2) as opool,
            tc.tile_pool(name="ot", bufs=2) as otpool,
        ):
            # weights cast f32 -> bf16 during SWDGE DMA
            wbd = cpool.tile([128, 896], bf16, tag="wbd")
            nc.gpsimd.dma_start(wbd[:, :], wbd_ext[:, :])

            def wsl(i):
                return wbd[:, 128 * i : 128 * i + 128]

            for s in range(n_slabs):
                x_sb = xpool.tile([128, 2048], bf16, tag="x")
                nc.gpsimd.dma_start(x_sb[:, :], x_r[s])

                xt = xtpool.tile([128, 2048], bf16, tag="xt")
                nc.vector.transpose(xt[:, :], x_sb[:, :])
                # xt[32*pg + f_, 64*n + 32*fb + b] = x[row(pg,b,n), 32*fb + f_]
                xt_v = xt[:, :].rearrange("p (n fb b) -> p n fb b", fb=2, b=32)

                # Layer 1: 64 -> 32, two accumulated K=32 pieces per chunk,
                # all 4 chunks in one block-diagonal K=128 matmul
                ps = hpsp.tile([128, 1024], f32, tag="hps")
                for hh in range(2):
                    for fb in range(2):
                        nc.tensor.matmul(
                            ps[:, 512 * hh : 512 * hh + 512],
                            lhsT=wsl(fb),
                            rhs=xt_v[:, 16 * hh : 16 * hh + 16, fb, :],
                            start=(fb == 0),
                            stop=(fb == 1),
                        )
                hprev = hpool.tile([128, 1024], bf16, tag="h")
                if act_split[0]:
                    nc.scalar.activation(hprev[:, :], ps[:, :], Relu)
                else:
                    nc.vector.tensor_scalar_max(hprev[:, :], ps[:, :], 0.0)

                # Layers 2..5: block-diag 32 -> 32
                for l in range(4):
                    ps = hpsp.tile([128, 1024], f32, tag="hps")
                    for hh in range(2):
                        nc.tensor.matmul(
                            ps[:, 512 * hh : 512 * hh + 512],
                            lhsT=wsl(2 + l),
                            rhs=hprev[:, 512 * hh : 512 * hh + 512],
                            start=True,
                            stop=True,
                        )
                    hnew = hpool.tile([128, 1024], bf16, tag="h")
                    if act_split[l + 1]:
                        nc.scalar.activation(hnew[:, :], ps[:, :], Relu)
                    else:
                        nc.vector.tensor_scalar_max(hnew[:, :], ps[:, :], 0.0)
                    hprev = hnew

                # Layer 6: block-diag 32 -> 24 (padded to 32 with zero cols)
                ops_t = opsp.tile([128, 1024], f32, tag="ops")
                for hh in range(2):
                    nc.tensor.matmul(
                        ops_t[:, 512 * hh : 512 * hh + 512],
                        lhsT=wsl(6),
                        rhs=hprev[:, 512 * hh : 512 * hh + 512],
                        start=True,
                        stop=True,
                    )
                o_sb = opool.tile([128, 1024], bf16, tag="ob")
                nc.scalar.copy(o_sb[:, :], ops_t[:, :])
                ot = otpool.tile([128, 1024], bf16, tag="ot")
                nc.vector.transpose(ot[:, :], o_sb[:, :])
                nc.gpsimd.dma_start(o_r[s], ot[:, :])
    nc.compile()
    return nc


def prep_weights(input_weight, hidden_weights, output_weights):
    """Build the 7 block-diagonal 128x128 stationaries, concat to [128, 896]."""
    hid_filter = np.kron(np.eye(4, dtype=np.float32), np.ones((8, 8), np.float32))
    out_filter = np.kron(np.eye(8, dtype=np.float32), np.ones((4, 3), np.float32))
    whm = hid_filter[None] * np.asarray(hidden_weights, np.float32)  # [4,32,32]
    wom = out_filter * np.asarray(output_weights, np.float32)  # [32,24]
    w_in = np.asarray(input_weight, np.float32)  # [64,32]

    mats = []
    for fb in range(2):
        mats.append(np.kron(np.eye(4, dtype=np.float32), w_in[32 * fb : 32 * fb + 32]))
    for l in range(4):
        mats.append(np.kron(np.eye(4, dtype=np.float32), whm[l]))
    wo_pad = np.zeros((32, 32), np.float32)
    wo_pad[:, :24] = wom
    mats.append(np.kron(np.eye(4, dtype=np.float32), wo_pad))
    return np.concatenate(mats, axis=1)  # [128, 7*128]


def kernel(x, input_weight, hidden_weights, output_weights):
    from concourse.bass_utils import run_bass_kernel_spmd

    x = np.ascontiguousarray(np.asarray(x, np.float32))
    wbd = prep_weights(input_weight, hidden_weights, output_weights)

    nc = build_nc(R)
    shards = x.reshape(N_CORES, R, 64)
    in_maps = [{"x": shards[i], "wbd": wbd} for i in range(N_CORES)]
    res = run_bass_kernel_spmd(nc, in_maps, core_ids=list(range(N_CORES)))
    outs = [np.asarray(res.results[i]["out"])[:, :24] for i in range(N_CORES)]
    return np.concatenate(outs, axis=0)


# revision 18
# speedup vs baseline: 3.5106x; 1.4045x over previous
"""Trainium2 Bass kernel for nn_BD dense MLP (block-diagonal hidden layers).

Network: x[B,64] -> relu(x@W_in)[B,32] -> 4x relu(h@(mask*W_h))[B,32]
         -> h@(mask*W_out)[B,24]

Strategy (pure data parallel over 8 cores, B=1048576, R=131072 rows/core):
 - x loaded batch-major contiguously; DVE 32x32 block-transpose flips each
   [32 batch x 32 feat] block to feature-major. The resulting batch
   permutation is undone by the output DMA access pattern.
 - All matmuls run feature-major: weights stationary (lhsT), activations
   moving (rhs, N=512). 4 batch chunks sit at partition groups 0..3 and use
   diagonal PE tile positions (rows 32c, cols 32c) so 4 matmuls stream
   concurrently in different 32x32 sub-arrays.
 - ReLU fused into the PSUM->SBUF move on ScalarE/VectorE at full 128
   partitions.
 - Output written padded [R,32]; host strips to 24 cols.
"""

import sys

import numpy as np

if "/opt/trn_rl_repo" not in sys.path:
    sys.path.insert(0, "/opt/trn_rl_repo")

N_CORES = 8
B_FULL = 1048576
R = B_FULL // N_CORES  # rows per core
SLAB = 4096  # rows per pipeline iteration
F32 = None  # set after import


def build_nc(rows=R, act_split=(True, True, True, True, False)):
    """Build the single-core SPMD Bass graph.

    act_split[l]: True -> relu on ScalarE, False -> relu on VectorE.
    """
    import concourse.bass as bass  # noqa: F401
    import concourse.mybir as mybir
    from concourse import bacc, tile

    f32 = mybir.dt.float32
    bf16 = mybir.dt.bfloat16
    nc = bacc.Bacc(None)

    x_ext = nc.declare_dram_parameter("x", [rows, 64], f32, isOutput=False)
    # 7 block-diagonal 128x128 stationaries: L1 fb0, L1 fb1, L2..L5, L6
    wbd_ext = nc.declare_dram_parameter("wbd", [128, 896], f32, isOutput=False)
    out_ext = nc.declare_dram_parameter("out", [rows, 32], f32, isOutput=True)

    n_slabs = rows // SLAB
    # x row r = s*4096 + p*32 + n  (p = SBUF partition, n = 0..31)
    x_r = x_ext.rearrange("(s p n) f -> s p (n f)", p=128, n=32)
    # out row r = s*4096 + pg*1024 + b*32 + n ; partition = 32*pg + b
    o_r = out_ext.rearrange("(s pg b n) c -> s (pg b) (n c)", pg=4, b=32, n=32)

    Relu = mybir.ActivationFunctionType.Relu

    NILV = 4  # slabs processed in interleaved groups
    assert n_slabs % NILV == 0 or n_slabs < NILV

    with tile.TileContext(nc) as tc:
        with (
            tc.tile_pool(name="const", bufs=1) as cpool,
            tc.tile_pool(name="xin", bufs=2 * NILV) as xpool,
            tc.tile_pool(name="xt", bufs=NILV + 1) as xtpool,
            tc.tile_pool(name="h", bufs=NILV + 2) as hpool,
            tc.tile_pool(name="ps", bufs=3, space="PSUM") as pspool,
            tc.tile_pool(name="ob", bufs=3) as opool,
            tc.tile_pool(name="ot", bufs=3) as otpool,
        ):
            # weights cast f32 -> bf16 during SWDGE DMA
            wbd = cpool.tile([128, 896], bf16, tag="wbd")
            nc.gpsimd.dma_start(wbd[:, :], wbd_ext[:, :])

            def wsl(i):
                return wbd[:, 128 * i : 128 * i + 128]

            def relu(out_t, in_t, on_act):
                if on_act:
                    nc.scalar.activation(out_t, in_t, Relu)
                else:
                    nc.vector.tensor_scalar_max(out_t, in_t, 0.0)

            group = NILV if n_slabs >= NILV else n_slabs
            for g0 in range(0, n_slabs, group):
                sl_ids = list(range(g0, min(g0 + group, n_slabs)))
                st = {s: {} for s in sl_ids}

                # stage: load + transpose x
                for s in sl_ids:
                    x_sb = xpool.tile([128, 2048], bf16, tag="x")
                    nc.gpsimd.dma_start(x_sb[:, :], x_r[s])
                    st[s]["x"] = x_sb
                for s in sl_ids:
                    xt = xtpool.tile([128, 2048], bf16, tag="xt")
                    nc.vector.transpose(xt[:, :], st[s]["x"][:, :])
                    st[s]["xt"] = xt[:, :].rearrange(
                        "p (n fb b) -> p n fb b", fb=2, b=32
                    )

                # Layer 1 (two accumulated K=32 pieces, block-diag K=128)
                for s in sl_ids:
                    ps = pspool.tile([128, 1024], f32, tag="ps")
                    for hh in range(2):
                        for fb in range(2):
                            nc.tensor.matmul(
                                ps[:, 512 * hh : 512 * hh + 512],
                                lhsT=wsl(fb),
                                rhs=st[s]["xt"][:, 16 * hh : 16 * hh + 16, fb, :],
                                start=(fb == 0),
                                stop=(fb == 1),
                            )
                    h = hpool.tile([128, 1024], bf16, tag="h")
                    relu(h[:, :], ps[:, :], True)
                    st[s]["h"] = h

                # Layers 2..5
                for l in range(4):
                    on_act = l < 3  # relu5 on DVE
                    for s in sl_ids:
                        ps = pspool.tile([128, 1024], f32, tag="ps")
                        for hh in range(2):
                            nc.tensor.matmul(
                                ps[:, 512 * hh : 512 * hh + 512],
                                lhsT=wsl(2 + l),
                                rhs=st[s]["h"][:, 512 * hh : 512 * hh + 512],
                                start=True,
                                stop=True,
                            )
                        h = hpool.tile([128, 1024], bf16, tag="h")
                        relu(h[:, :], ps[:, :], on_act)
                        st[s]["h"] = h

                # Layer 6 + copy + transpose-back + store
                for s in sl_ids:
                    ps = pspool.tile([128, 1024], f32, tag="ps")
                    for hh in range(2):
                        nc.tensor.matmul(
                            ps[:, 512 * hh : 512 * hh + 512],
                            lhsT=wsl(6),
                            rhs=st[s]["h"][:, 512 * hh : 512 * hh + 512],
                            start=True,
                            stop=True,
                        )
                    o_sb = opool.tile([128, 1024], bf16, tag="ob")
                    if s % 3 == 2:
                        nc.vector.tensor_copy(o_sb[:, :], ps[:, :])
                    else:
                        nc.scalar.copy(o_sb[:, :], ps[:, :])
                    ot = otpool.tile([128, 1024], bf16, tag="ot")
                    nc.vector.transpose(ot[:, :], o_sb[:, :])
                    nc.gpsimd.dma_start(o_r[s], ot[:, :])
    nc.compile()
    return nc


def prep_weights(input_weight, hidden_weights, output_weights):
    """Build the 7 block-diagonal 128x128 stationaries, concat to [128, 896]."""
    hid_filter = np.kron(np.eye(4, dtype=np.float32), np.ones((8, 8), np.float32))
    out_filter = np.kron(np.eye(8, dtype=np.float32), np.ones((4, 3), np.float32))
    whm = hid_filter[None] * np.asarray(hidden_weights, np.float32)  # [4,32,32]
    wom = out_filter * np.asarray(output_weights, np.float32)  # [32,24]
    w_in = np.asarray(input_weight, np.float32)  # [64,32]

    mats = []
    for fb in range(2):
        mats.append(np.kron(np.eye(4, dtype=np.float32), w_in[32 * fb : 32 * fb + 32]))
    for l in range(4):
        mats.append(np.kron(np.eye(4, dtype=np.float32), whm[l]))
    wo_pad = np.zeros((32, 32), np.float32)
    wo_pad[:, :24] = wom
    mats.append(np.kron(np.eye(4, dtype=np.float32), wo_pad))
    return np.concatenate(mats, axis=1)  # [128, 7*128]


def kernel(x, input_weight, hidden_weights, output_weights):
    from concourse.bass_utils import run_bass_kernel_spmd

    x = np.ascontiguousarray(np.asarray(x, np.float32))
    wbd = prep_weights(input_weight, hidden_weights, output_weights)

    nc = build_nc(R)
    shards = x.reshape(N_CORES, R, 64)
    in_maps = [{"x": shards[i], "wbd": wbd} for i in range(N_CORES)]
    res = run_bass_kernel_spmd(nc, in_maps, core_ids=list(range(N_CORES)))
    outs = [np.asarray(res.results[i]["out"])[:, :24] for i in range(N_CORES)]
    return np.concatenate(outs, axis=0)


# revision 23
# speedup vs baseline: 3.6571x; 1.0417x over previous
"""Trainium2 Bass kernel for nn_BD dense MLP (block-diagonal hidden layers).

Network: x[B,64] -> relu(x@W_in)[B,32] -> 4x relu(h@(mask*W_h))[B,32]
         -> h@(mask*W_out)[B,24]

Strategy (pure data parallel over 8 cores, B=1048576, R=131072 rows/core):
 - x loaded batch-major contiguously; DVE 32x32 block-transpose flips each
   [32 batch x 32 feat] block to feature-major. The resulting batch
   permutation is undone by the output DMA access pattern.
 - All matmuls run feature-major: weights stationary (lhsT), activations
   moving (rhs, N=512). 4 batch chunks sit at partition groups 0..3 and use
   diagonal PE tile positions (rows 32c, cols 32c) so 4 matmuls stream
   concurrently in different 32x32 sub-arrays.
 - ReLU fused into the PSUM->SBUF move on ScalarE/VectorE at full 128
   partitions.
 - Output written padded [R,32]; host strips to 24 cols.
"""

import sys

import numpy as np

if "/opt/trn_rl_repo" not in sys.path:
    sys.path.insert(0, "/opt/trn_rl_repo")

N_CORES = 8
B_FULL = 1048576
R = B_FULL // N_CORES  # rows per core
SLAB = 4096  # rows per pipeline iteration
F32 = None  # set after import


def build_nc(rows=R, act_split=(True, True, True, True, False)):
    """Build the single-core SPMD Bass graph.

    act_split[l]: True -> relu on ScalarE, False -> relu on VectorE.
    """
    import concourse.bass as bass  # noqa: F401
    import concourse.mybir as mybir
    from concourse import bacc, tile

    f32 = mybir.dt.float32
    bf16 = mybir.dt.bfloat16
    nc = bacc.Bacc(None)

    x_ext = nc.declare_dram_parameter("x", [rows, 64], f32, isOutput=False)
    # 7 block-diagonal 128x128 stationaries: L1 fb0, L1 fb1, L2..L5, L6
    wbd_ext = nc.declare_dram_parameter("wbd", [128, 896], f32, isOutput=False)
    out_ext = nc.declare_dram_parameter("out", [rows, 32], bf16, isOutput=True)

    n_slabs = rows // SLAB
    # x row r = s*4096 + p*32 + n  (p = SBUF partition, n = 0..31)
    x_r = x_ext.rearrange("(s p n) f -> s p (n f)", p=128, n=32)
    # out row r = s*4096 + pg*1024 + b*32 + n ; partition = 32*pg + b
    o_r = out_ext.rearrange("(s pg b n) c -> s (pg b) (n c)", pg=4, b=32, n=32)

    Relu = mybir.ActivationFunctionType.Relu

    NILV = 4  # slabs processed in interleaved groups
    assert n_slabs % NILV == 0 or n_slabs < NILV

    with tile.TileContext(nc) as tc:
        with (
            tc.tile_pool(name="const", bufs=1) as cpool,
            tc.tile_pool(name="xin", bufs=2 * NILV) as xpool,
            tc.tile_pool(name="xt", bufs=NILV + 1) as xtpool,
            tc.tile_pool(name="h", bufs=NILV + 2) as hpool,
            tc.tile_pool(name="ps", bufs=4, space="PSUM") as pspool,
            tc.tile_pool(name="ot", bufs=3) as otpool,
        ):
            # weights cast f32 -> bf16 during SWDGE DMA
            wbd = cpool.tile([128, 896], bf16, tag="wbd")
            nc.gpsimd.dma_start(wbd[:, :], wbd_ext[:, :])

            def wsl(i):
                return wbd[:, 128 * i : 128 * i + 128]

            def relu(out_t, in_t, on_act):
                if on_act:
                    nc.scalar.activation(out_t, in_t, Relu)
                else:
                    nc.vector.tensor_scalar_max(out_t, in_t, 0.0)

            group = NILV if n_slabs >= NILV else n_slabs
            for g0 in range(0, n_slabs, group):
                sl_ids = list(range(g0, min(g0 + group, n_slabs)))
                st = {s: {} for s in sl_ids}

                # stage: load + transpose x
                for s in sl_ids:
                    x_sb = xpool.tile([128, 2048], bf16, tag="x")
                    nc.gpsimd.dma_start(x_sb[:, :], x_r[s])
                    st[s]["x"] = x_sb
                for s in sl_ids:
                    xt = xtpool.tile([128, 2048], bf16, tag="xt")
                    nc.vector.transpose(xt[:, :], st[s]["x"][:, :])
                    st[s]["xt"] = xt[:, :].rearrange(
                        "p (n fb b) -> p n fb b", fb=2, b=32
                    )

                # Layer 1 (two accumulated K=32 pieces, block-diag K=128)
                for s in sl_ids:
                    ps = pspool.tile([128, 1024], f32, tag="ps")
                    for hh in range(2):
                        for fb in range(2):
                            nc.tensor.matmul(
                                ps[:, 512 * hh : 512 * hh + 512],
                                lhsT=wsl(fb),
                                rhs=st[s]["xt"][:, 16 * hh : 16 * hh + 16, fb, :],
                                start=(fb == 0),
                                stop=(fb == 1),
                            )
                    h = hpool.tile([128, 1024], bf16, tag="h")
                    relu(h[:, :], ps[:, :], True)
                    st[s]["h"] = h

                # Layers 2..5
                for l in range(4):
                    on_act = l < 3  # relu5 on DVE
                    for s in sl_ids:
                        ps = pspool.tile([128, 1024], f32, tag="ps")
                        for hh in range(2):
                            nc.tensor.matmul(
                                ps[:, 512 * hh : 512 * hh + 512],
                                lhsT=wsl(2 + l),
                                rhs=st[s]["h"][:, 512 * hh : 512 * hh + 512],
                                start=True,
                                stop=True,
                            )
                        h = hpool.tile([128, 1024], bf16, tag="h")
                        relu(h[:, :], ps[:, :], on_act)
                        st[s]["h"] = h

                # Layer 6 + copy + transpose-back + store
                for s in sl_ids:
                    ps = pspool.tile([128, 1024], f32, tag="ps")
                    for hh in range(2):
                        nc.tensor.matmul(
                            ps[:, 512 * hh : 512 * hh + 512],
                            lhsT=wsl(6),
                            rhs=st[s]["h"][:, 512 * hh : 512 * hh + 512],
                            start=True,
                            stop=True,
                        )
                    ot = otpool.tile([128, 1024], f32, tag="ot")
                    nc.vector.transpose(ot[:, :], ps[:, :])
                    nc.gpsimd.dma_start(o_r[s], ot[:, :])
    nc.compile()
    return nc


def prep_weights(input_weight, hidden_weights, output_weights):
    """Build the 7 block-diagonal 128x128 stationaries, concat to [128, 896]."""
    hid_filter = np.kron(np.eye(4, dtype=np.float32), np.ones((8, 8), np.float32))
    out_filter = np.kron(np.eye(8, dtype=np.float32), np.ones((4, 3), np.float32))
    whm = hid_filter[None] * np.asarray(hidden_weights, np.float32)  # [4,32,32]
    wom = out_filter * np.asarray(output_weights, np.float32)  # [32,24]
    w_in = np.asarray(input_weight, np.float32)  # [64,32]

    mats = []
    for fb in range(2):
        mats.append(np.kron(np.eye(4, dtype=np.float32), w_in[32 * fb : 32 * fb + 32]))
    for l in range(4):
        mats.append(np.kron(np.eye(4, dtype=np.float32), whm[l]))
    wo_pad = np.zeros((32, 32), np.float32)
    wo_pad[:, :24] = wom
    mats.append(np.kron(np.eye(4, dtype=np.float32), wo_pad))
    return np.concatenate(mats, axis=1)  # [128, 7*128]


def kernel(x, input_weight, hidden_weights, output_weights):
    from concourse.bass_utils import run_bass_kernel_spmd

    x = np.ascontiguousarray(np.asarray(x, np.float32))
    wbd = prep_weights(input_weight, hidden_weights, output_weights)

    nc = build_nc(R)
    shards = x.reshape(N_CORES, R, 64)
    in_maps = [{"x": shards[i], "wbd": wbd} for i in range(N_CORES)]
    res = run_bass_kernel_spmd(nc, in_maps, core_ids=list(range(N_CORES)))
    outs = [
        np.asarray(res.results[i]["out"]).astype(np.float32)[:, :24]
        for i in range(N_CORES)
    ]
    return np.concatenate(outs, axis=0)


# revision 25
# speedup vs baseline: 4.4148x; 1.2072x over previous
"""Trainium2 Bass kernel for nn_BD dense MLP (block-diagonal hidden layers).

Network: x[B,64] -> relu(x@W_in)[B,32] -> 4x relu(h@(mask*W_h))[B,32]
         -> h@(mask*W_out)[B,24]

Strategy (pure data parallel over 8 cores, B=1048576, R=131072 rows/core):
 - x loaded batch-major contiguously; DVE 32x32 block-transpose flips each
   [32 batch x 32 feat] block to feature-major. The resulting batch
   permutation is undone by the output DMA access pattern.
 - All matmuls run feature-major: weights stationary (lhsT), activations
   moving (rhs, N=512). 4 batch chunks sit at partition groups 0..3 and use
   diagonal PE tile positions (rows 32c, cols 32c) so 4 matmuls stream
   concurrently in different 32x32 sub-arrays.
 - ReLU fused into the PSUM->SBUF move on ScalarE/VectorE at full 128
   partitions.
 - Output written padded [R,32]; host strips to 24 cols.
"""

import sys

import numpy as np

if "/opt/trn_rl_repo" not in sys.path:
    sys.path.insert(0, "/opt/trn_rl_repo")

N_CORES = 8
B_FULL = 1048576
R = B_FULL // N_CORES  # rows per core
SLAB = 4096  # rows per pipeline iteration
F32 = None  # set after import


def build_nc(rows=R, act_split=(True, True, True, True, False)):
    """Build the single-core SPMD Bass graph.

    act_split[l]: True -> relu on ScalarE, False -> relu on VectorE.
    """
    import concourse.bass as bass  # noqa: F401
    import concourse.mybir as mybir
    from concourse import bacc, tile

    f32 = mybir.dt.float32
    bf16 = mybir.dt.bfloat16
    nc = bacc.Bacc(None)

    x_ext = nc.declare_dram_parameter("x", [rows, 64], f32, isOutput=False)
    # 7 block-diagonal 128x128 stationaries: L1 fb0, L1 fb1, L2..L5, L6
    wbd_ext = nc.declare_dram_parameter("wbd", [128, 896], f32, isOutput=False)
    out_ext = nc.declare_dram_parameter("out", [rows, 32], f32, isOutput=True)

    n_slabs = rows // SLAB
    # x row r = s*4096 + p*32 + n  (p = SBUF partition, n = 0..31)
    x_r = x_ext.rearrange("(s p n) f -> s p (n f)", p=128, n=32)
    # out row r = s*4096 + pg*1024 + b*32 + n ; partition = 32*pg + b
    o_r = out_ext.rearrange("(s pg b n) c -> s (pg b) (n c)", pg=4, b=32, n=32)

    Relu = mybir.ActivationFunctionType.Relu

    NILV = 4  # slabs processed in interleaved groups
    assert n_slabs % NILV == 0 or n_slabs < NILV

    with tile.TileContext(nc) as tc:
        with (
            tc.tile_pool(name="const", bufs=1) as cpool,
            tc.tile_pool(name="xin", bufs=2 * NILV) as xpool,
            tc.tile_pool(name="xt", bufs=NILV + 1) as xtpool,
            tc.tile_pool(name="h", bufs=NILV + 2) as hpool,
            tc.tile_pool(name="ps", bufs=4, space="PSUM") as pspool,
            tc.tile_pool(name="ot", bufs=3) as otpool,
        ):
            # weights cast f32 -> bf16 during SWDGE DMA
            wbd = cpool.tile([128, 896], bf16, tag="wbd")
            nc.gpsimd.dma_start(wbd[:, :], wbd_ext[:, :])

            def wsl(i):
                return wbd[:, 128 * i : 128 * i + 128]

            def relu(out_t, in_t, on_act):
                if on_act:
                    nc.scalar.activation(out_t, in_t, Relu)
                else:
                    nc.vector.tensor_scalar_max(out_t, in_t, 0.0)

            group = NILV if n_slabs >= NILV else n_slabs
            for g0 in range(0, n_slabs, group):
                sl_ids = list(range(g0, min(g0 + group, n_slabs)))
                st = {s: {} for s in sl_ids}

                # stage: load + transpose x
                for s in sl_ids:
                    x_sb = xpool.tile([128, 2048], bf16, tag="x")
                    nc.gpsimd.dma_start(x_sb[:, :], x_r[s])
                    st[s]["x"] = x_sb
                for s in sl_ids:
                    xt = xtpool.tile([128, 2048], bf16, tag="xt")
                    nc.vector.transpose(xt[:, :], st[s]["x"][:, :])
                    st[s]["xt"] = xt[:, :].rearrange(
                        "p (n fb b) -> p n fb b", fb=2, b=32
                    )

                # Layer 1 (two accumulated K=32 pieces, block-diag K=128)
                for s in sl_ids:
                    ps = pspool.tile([128, 1024], f32, tag="ps")
                    for hh in range(2):
                        for fb in range(2):
                            nc.tensor.matmul(
                                ps[:, 512 * hh : 512 * hh + 512],
                                lhsT=wsl(fb),
                                rhs=st[s]["xt"][:, 16 * hh : 16 * hh + 16, fb, :],
                                start=(fb == 0),
                                stop=(fb == 1),
                            )
                    h = hpool.tile([128, 1024], bf16, tag="h")
                    relu(h[:, :], ps[:, :], True)
                    st[s]["h"] = h

                # Layers 2..5
                for l in range(4):
                    on_act = l < 3  # relu5 on DVE
                    for s in sl_ids:
                        ps = pspool.tile([128, 1024], f32, tag="ps")
                        for hh in range(2):
                            nc.tensor.matmul(
                                ps[:, 512 * hh : 512 * hh + 512],
                                lhsT=wsl(2 + l),
                                rhs=st[s]["h"][:, 512 * hh : 512 * hh + 512],
                                start=True,
                                stop=True,
                            )
                        h = hpool.tile([128, 1024], bf16, tag="h")
                        relu(h[:, :], ps[:, :], on_act)
                        st[s]["h"] = h

                # Layer 6 + copy + transpose-back + store
                for s in sl_ids:
                    ps = pspool.tile([128, 1024], f32, tag="ps")
                    for hh in range(2):
                        nc.tensor.matmul(
                            ps[:, 512 * hh : 512 * hh + 512],
                            lhsT=wsl(6),
                            rhs=st[s]["h"][:, 512 * hh : 512 * hh + 512],
                            start=True,
                            stop=True,
                        )
                    ot = otpool.tile([128, 1024], f32, tag="ot")
                    nc.vector.transpose(ot[:, :], ps[:, :])
                    nc.sync.dma_start(o_r[s], ot[:, :])
    nc.compile()
    return nc


def prep_weights(input_weight, hidden_weights, output_weights):
    """Build the 7 block-diagonal 128x128 stationaries, concat to [128, 896]."""
    hid_filter = np.kron(np.eye(4, dtype=np.float32), np.ones((8, 8), np.float32))
    out_filter = np.kron(np.eye(8, dtype=np.float32), np.ones((4, 3), np.float32))
    whm = hid_filter[None] * np.asarray(hidden_weights, np.float32)  # [4,32,32]
    wom = out_filter * np.asarray(output_weights, np.float32)  # [32,24]
    w_in = np.asarray(input_weight, np.float32)  # [64,32]

    mats = []
    for fb in range(2):
        mats.append(np.kron(np.eye(4, dtype=np.float32), w_in[32 * fb : 32 * fb + 32]))
    for l in range(4):
        mats.append(np.kron(np.eye(4, dtype=np.float32), whm[l]))
    wo_pad = np.zeros((32, 32), np.float32)
    wo_pad[:, :24] = wom
    mats.append(np.kron(np.eye(4, dtype=np.float32), wo_pad))
    return np.concatenate(mats, axis=1)  # [128, 7*128]


def kernel(x, input_weight, hidden_weights, output_weights):
    from concourse.bass_utils import run_bass_kernel_spmd

    x = np.ascontiguousarray(np.asarray(x, np.float32))
    wbd = prep_weights(input_weight, hidden_weights, output_weights)

    nc = build_nc(R)
    shards = x.reshape(N_CORES, R, 64)
    in_maps = [{"x": shards[i], "wbd": wbd} for i in range(N_CORES)]
    res = run_bass_kernel_spmd(nc, in_maps, core_ids=list(range(N_CORES)))
    outs = [
        np.asarray(res.results[i]["out"]).astype(np.float32)[:, :24]
        for i in range(N_CORES)
    ]
    return np.concatenate(outs, axis=0)


# revision 29
# speedup vs baseline: 4.5043x; 1.0203x over previous
"""Trainium2 Bass kernel for nn_BD dense MLP (block-diagonal hidden layers).

Network: x[B,64] -> relu(x@W_in)[B,32] -> 4x relu(h@(mask*W_h))[B,32]
         -> h@(mask*W_out)[B,24]

Strategy (pure data parallel over 8 cores, B=1048576, R=131072 rows/core):
 - x loaded batch-major contiguously; DVE 32x32 block-transpose flips each
   [32 batch x 32 feat] block to feature-major. The resulting batch
   permutation is undone by the output DMA access pattern.
 - All matmuls run feature-major: weights stationary (lhsT), activations
   moving (rhs, N=512). 4 batch chunks sit at partition groups 0..3 and use
   diagonal PE tile positions (rows 32c, cols 32c) so 4 matmuls stream
   concurrently in different 32x32 sub-arrays.
 - ReLU fused into the PSUM->SBUF move on ScalarE/VectorE at full 128
   partitions.
 - Output written padded [R,32]; host strips to 24 cols.
"""

import sys

import numpy as np

if "/opt/trn_rl_repo" not in sys.path:
    sys.path.insert(0, "/opt/trn_rl_repo")

N_CORES = 8
B_FULL = 1048576
R = B_FULL // N_CORES  # rows per core
SLAB = 4096  # rows per pipeline iteration
F32 = None  # set after import


def build_nc(rows=R, act_split=(True, True, True, True, False)):
    """Build the single-core SPMD Bass graph.

    act_split[l]: True -> relu on ScalarE, False -> relu on VectorE.
    """
    import concourse.bass as bass  # noqa: F401
    import concourse.mybir as mybir
    from concourse import bacc, tile

    f32 = mybir.dt.float32
    bf16 = mybir.dt.bfloat16
    nc = bacc.Bacc(None)

    x_ext = nc.declare_dram_parameter("x", [rows, 64], bf16, isOutput=False)
    # 7 block-diagonal 128x128 stationaries: L1 fb0, L1 fb1, L2..L5, L6
    wbd_ext = nc.declare_dram_parameter("wbd", [128, 896], bf16, isOutput=False)
    out_ext = nc.declare_dram_parameter("out", [rows, 32], f32, isOutput=True)

    n_slabs = rows // SLAB
    # x row r = s*4096 + p*32 + n  (p = SBUF partition, n = 0..31)
    x_r = x_ext.rearrange("(s p n) f -> s p (n f)", p=128, n=32)
    # out row r = s*4096 + pg*1024 + b*32 + n ; partition = 32*pg + b
    o_r = out_ext.rearrange("(s pg b n) c -> s (pg b) (n c)", pg=4, b=32, n=32)

    Relu = mybir.ActivationFunctionType.Relu

    NILV = 4  # slabs processed in interleaved groups
    assert n_slabs % NILV == 0 or n_slabs < NILV

    with tile.TileContext(nc) as tc:
        with (
            tc.tile_pool(name="const", bufs=1) as cpool,
            tc.tile_pool(name="xin", bufs=2 * NILV) as xpool,
            tc.tile_pool(name="xt", bufs=NILV + 1) as xtpool,
            tc.tile_pool(name="h", bufs=NILV + 2) as hpool,
            tc.tile_pool(name="ps", bufs=4, space="PSUM") as pspool,
            tc.tile_pool(name="ot", bufs=3) as otpool,
        ):
            wbd = cpool.tile([128, 896], bf16, tag="wbd")
            nc.sync.dma_start(wbd[:, :], wbd_ext[:, :])

            def wsl(i):
                return wbd[:, 128 * i : 128 * i + 128]

            def relu(out_t, in_t, on_act):
                if on_act:
                    nc.scalar.activation(out_t, in_t, Relu)
                else:
                    nc.vector.tensor_scalar_max(out_t, in_t, 0.0)

            group = NILV if n_slabs >= NILV else n_slabs
            for g0 in range(0, n_slabs, group):
                sl_ids = list(range(g0, min(g0 + group, n_slabs)))
                st = {s: {} for s in sl_ids}

                # stage: load + transpose x
                for s in sl_ids:
                    x_sb = xpool.tile([128, 2048], bf16, tag="x")
                    nc.sync.dma_start(x_sb[:, :], x_r[s])
                    st[s]["x"] = x_sb
                for s in sl_ids:
                    xt = xtpool.tile([128, 2048], bf16, tag="xt")
                    nc.vector.transpose(xt[:, :], st[s]["x"][:, :])
                    st[s]["xt"] = xt[:, :].rearrange(
                        "p (n fb b) -> p n fb b", fb=2, b=32
                    )

                # Layer 1 (two accumulated K=32 pieces, block-diag K=128)
                for s in sl_ids:
                    ps = pspool.tile([128, 1024], f32, tag="ps")
                    for hh in range(2):
                        for fb in range(2):
                            nc.tensor.matmul(
                                ps[:, 512 * hh : 512 * hh + 512],
                                lhsT=wsl(fb),
                                rhs=st[s]["xt"][:, 16 * hh : 16 * hh + 16, fb, :],
                                start=(fb == 0),
                                stop=(fb == 1),
                            )
                    h = hpool.tile([128, 1024], bf16, tag="h")
                    relu(h[:, :], ps[:, :], True)
                    st[s]["h"] = h

                # Layers 2..5
                for l in range(4):
                    on_act = l < 3  # relu5 on DVE
                    for s in sl_ids:
                        ps = pspool.tile([128, 1024], f32, tag="ps")
                        for hh in range(2):
                            nc.tensor.matmul(
                                ps[:, 512 * hh : 512 * hh + 512],
                                lhsT=wsl(2 + l),
                                rhs=st[s]["h"][:, 512 * hh : 512 * hh + 512],
                                start=True,
                                stop=True,
                            )
                        h = hpool.tile([128, 1024], bf16, tag="h")
                        relu(h[:, :], ps[:, :], on_act)
                        st[s]["h"] = h

                # Layer 6 + copy + transpose-back + store
                for s in sl_ids:
                    ps = pspool.tile([128, 1024], f32, tag="ps")
                    for hh in range(2):
                        nc.tensor.matmul(
                            ps[:, 512 * hh : 512 * hh + 512],
                            lhsT=wsl(6),
                            rhs=st[s]["h"][:, 512 * hh : 512 * hh + 512],
                            start=True,
                            stop=True,
                        )
                    ot = otpool.tile([128, 1024], f32, tag="ot")
                    nc.vector.transpose(ot[:, :], ps[:, :])
                    nc.sync.dma_start(o_r[s], ot[:, :])
    nc.compile()
    return nc


def prep_weights(input_weight, hidden_weights, output_weights):
    """Build the 7 block-diagonal 128x128 stationaries, concat to [128, 896]."""
    hid_filter = np.kron(np.eye(4, dtype=np.float32), np.ones((8, 8), np.float32))
    out_filter = np.kron(np.eye(8, dtype=np.float32), np.ones((4, 3), np.float32))
    whm = hid_filter[None] * np.asarray(hidden_weights, np.float32)  # [4,32,32]
    wom = out_filter * np.asarray(output_weights, np.float32)  # [32,24]
    w_in = np.asarray(input_weight, np.float32)  # [64,32]

    mats = []
    for fb in range(2):
        mats.append(np.kron(np.eye(4, dtype=np.float32), w_in[32 * fb : 32 * fb + 32]))
    for l in range(4):
        mats.append(np.kron(np.eye(4, dtype=np.float32), whm[l]))
    wo_pad = np.zeros((32, 32), np.float32)
    wo_pad[:, :24] = wom
    mats.append(np.kron(np.eye(4, dtype=np.float32), wo_pad))
    return np.concatenate(mats, axis=1)  # [128, 7*128]


def to_bf16(a):
    import ml_dtypes

    return np.asarray(a, np.float32).astype(ml_dtypes.bfloat16)


def kernel(x, input_weight, hidden_weights, output_weights):
    from concourse.bass_utils import run_bass_kernel_spmd

    x = to_bf16(x)
    wbd = to_bf16(prep_weights(input_weight, hidden_weights, output_weights))

    nc = build_nc(R)
    shards = x.reshape(N_CORES, R, 64)
    in_maps = [{"x": shards[i], "wbd": wbd} for i in range(N_CORES)]
    res = run_bass_kernel_spmd(nc, in_maps, core_ids=list(range(N_CORES)))
    outs = [
        np.asarray(res.results[i]["out"]).astype(np.float32)[:, :24]
        for i in range(N_CORES)
    ]
    return np.concatenate(outs, axis=0)
